# revision 1
# baseline (speedup 1.0000x reference)
_C8_B64 = "Cx2VQ76vfEN9P3dDjox0QxvMu0LnF79CNUC7QlBZukJE87hC/za9QjWCuELPS71C+Vq9QvaEuEIBfLlC4yK7QnDWtkJq3rlCHuu3Qg+LvULzrrZCLE65QuJft0I5trtCxZe2QqCvukI3erdC+Ce3QmALu0JrzbRC0Je3QjPAvUI/DLdCEd26QrIDuUKw+bxCdxq2QuiKuUJ7rrlCmt+7QqcJu0KYFLdCsFi7Qnpst0JC2bVCUT+5QhUEuEJPTbpCcou3Qhdbt0J1wrdC04e1QjosvkJ+jrdCKLG5QmRYvUJ93LVCZj25Qia5tUIiu7tCmj21Qo1yt0IxSb1CCxi7Qq/WtkJpC7hC42q1QmrKukKCxLtCQoa3Qksxt0IpqLdC6v62QrrfuEL2mrpCW/K4QjCsvELZ5LdC3kO5QmirukIm+rlCVay4QrtLt0JqFLlC5Dy6QvQFt0LBXLlCyw64Qogru0IOJblCv8a5QrkyuELxGLpCjMm0Qm75u0LGrLpC/2S1QgXNuEKTXrhCaa+9QmWat0Kl8rpC6Da3QkqUukJj1rlCNAW3QmervkLWbbdCSxbAQkodtULEzb9CuJO5QnlBu0KK2rhCmni9QqSbt0IJqsBCRZO5QuGYvUJsZbpC3mi9QvbtvUKPbbxCn2C7QhqevkJ8Q75C6rm5QiySwEI="
"""CTC batch cost (keras ctc_batch_cost semantics) on 8 Trainium2 NeuronCores.

Strategy (pure data parallel, 32 examples per core):
  The CTC forward recurrence alpha_t[s] = p_t[s]*(alpha[s]+alpha[s-1]+m[s]*alpha[s-2])
  is computed lane-by-lane over extended-label states s (lane s only needs the full
  time series of lanes s-1, s-2), so each lane is ONE hardware `tensor_tensor_scan`
  over t with fp32 internal state: state = (d0 + state) * d1.
  Numerical range is handled by an offline-tuned per-8-step scale schedule folded
  into the on-device bf16 cast, plus a static pair tilt sigma[s] = V2^ceil(s/2) *
  alpha[s] (V2 = e^-4.5, applied on hops into odd lanes) that flattens the
  cross-lane dynamic range into f32/bf16. Even (blank) lanes need zero prep ops:
  their scan reads the previous lane's stored series directly.
  The label-class gather p_t[s] is a one-hot matmul on the tensor engine against
  xbar-DMA-transposed bf16 y_pred (bounced through DRAM scratch, 8 examples per
  transpose to amortize xbar-mode switch serialization).
"""
import base64
import numpy as np
import ml_dtypes

B, T, C, L = 256, 1024, 96, 64
S = 2 * L + 1  # 129
BLANK = C - 1
EPS = 1e-7
NCORES = 8
BPC = B // NCORES  # 32 examples per core
NLANE = L + 1  # 64 label lanes + blank lane in the gather output
NBLK = 128  # schedule blocks (8 t-steps each)
GRP = 8  # examples per merged xbar transpose

G = -2.25
V2 = np.float32(np.exp(2.0 * G))  # odd-hop tilt factor

C8 = np.frombuffer(base64.b64decode(_C8_B64), dtype=np.float32).copy()  # [128]
C_SCHED = np.repeat(C8, 8)  # [T]
C8_EPS = (C8.astype(np.float64) * EPS).astype(np.float32)
K_CORR = float(np.sum(np.log(C_SCHED.astype(np.float64))))
K_FIN = float(64.0 * np.log(np.float64(V2)) + K_CORR - 64.0 * np.log(2.0))

_PROGRAM = None


def _build_program(debug_taps=False, phase=None):
    import concourse.bacc as bacc
    import concourse.tile as tile
    import concourse.mybir as mybir

    f32 = mybir.dt.float32
    bf = mybir.dt.bfloat16
    ADD = mybir.AluOpType.add
    MULT = mybir.AluOpType.mult

    nc = bacc.Bacc("TRN2", target_bir_lowering=False, debug=False, num_devices=NCORES)
    yp_d = nc.dram_tensor("y_pred", [BPC, T, C], f32, kind="ExternalInput")
    e_d = nc.dram_tensor("onehot", [C, BPC * NLANE], bf, kind="ExternalInput")
    m_d = nc.dram_tensor("mv2", [BPC, L], f32, kind="ExternalInput")
    cs_d = nc.dram_tensor("c8pair", [NBLK, 2], f32, kind="ExternalInput")
    out_d = nc.dram_tensor("out", [BPC, 1], f32, kind="ExternalOutput")
    gbd = nc.dram_tensor("gbd", [BPC, NLANE * T], bf)  # gather-output DRAM bounce
    if debug_taps:
        dbg_lanes = nc.dram_tensor("dbg_lanes", [S, BPC, T + 1], bf, kind="ExternalOutput")

    with tile.TileContext(nc) as tc:
        with (
            tc.tile_pool(name="const", bufs=1) as const_pool,
            tc.tile_pool(name="stage", bufs=3) as stage_pool,
            tc.tile_pool(name="scr", bufs=4, space="DRAM") as scr_pool,
            tc.tile_pool(name="ypt", bufs=2) as ypt_pool,
            tc.tile_pool(name="ps", bufs=4, space="PSUM") as psum_pool,
            tc.tile_pool(name="gb", bufs=3) as gb_pool,
            tc.tile_pool(name="big", bufs=1) as big_pool,
            tc.tile_pool(name="w", bufs=2) as w_pool,
            tc.tile_pool(name="d0", bufs=2) as d_pool,
            tc.tile_pool(name="fin", bufs=1) as fin_pool,
        ):
            # ---- constants ----
            cs_sb = const_pool.tile([NBLK, 2], f32, tag="cs")
            nc.sync.dma_start(cs_sb[:], cs_d.ap())
            e_sb = const_pool.tile([C, BPC * NLANE], bf, tag="E")
            nc.sync.dma_start(e_sb[:], e_d.ap())
            m_sb = const_pool.tile([BPC, L], f32, tag="m")
            nc.sync.dma_start(m_sb[:], m_d.ap())

            ylab = big_pool.tile([BPC, NLANE * T], bf, tag="ylab")
            # 4 static alpha tiles: index 0 for lane 0 (boundary 1.0), 1..3 rotate.
            atiles = [
                big_pool.tile([BPC, T + 1], bf, tag=f"a{i}", name=f"atile{i}")
                for i in range(4)
            ]
            nc.gpsimd.memset(atiles[0][:, 0:1], 1.0)
            for i in range(1, 4):
                nc.gpsimd.memset(atiles[i][:, 0:1], 0.0)

            ypa = yp_d.ap()

            # 3 static cast tiles, fully zeroed once so the pad columns are
            # initialized for the full-tile scratch store.
            tcasts = [
                big_pool.tile([128, 1024], bf, tag=f"tc{i}", name=f"tcast{i}")
                for i in range(3)
            ]
            for tt in tcasts:
                nc.gpsimd.memset(tt[:], 0.0)

            # ---- gather phase A: load -> scaled bf16 cast -> DRAM scratch ----
            gather_bs = range(BPC) if phase != "scan" else range(0)
            if phase == "scan":
                nc.gpsimd.memset(ylab[:], 0.25)
            scrs = {}
            for g in range(0, BPC // GRP if phase != "scan" else 0):
                scrs[g] = scr_pool.tile([GRP * T, 128], bf, tag="scr", name=f"scr{g}")
            last_store = None
            for b in gather_bs:
                tin = stage_pool.tile([128, 8 * C], f32, tag="in")
                nc.sync.dma_start(tin[:], ypa[b].rearrange("(p k) c -> p (k c)", p=128))
                tcast = tcasts[b % 3]
                # out view: 8 blocks of 96 (stride 128), matching scratch row layout
                nc.scalar.activation(
                    tcast[:].rearrange("p (k c) -> p k c", k=8)[:, :, 0:C],
                    tin[:].rearrange("p (k c) -> p k c", k=8),
                    mybir.ActivationFunctionType.Identity,
                    bias=cs_sb[:, 1:2],
                    scale=cs_sb[:, 0:1],
                )
                last_store = nc.scalar.dma_start(
                    scrs[b // GRP][(b % GRP) * T : (b % GRP + 1) * T, :], tcast[:]
                )

            # ---- gather phase B: xbar transpose -> one-hot matmul -> bounce ----
            for g in range(0, (BPC // GRP) if phase != "scan" else 0):
                ypt = ypt_pool.tile([128, GRP * T], bf, tag="ypt")
                tr = nc.sync.dma_start(ypt[:], scrs[g][:], transpose=True)
                for bl in range(GRP):
                    b = g * GRP + bl
                    ps = psum_pool.tile([NLANE, T], f32, tag="ps")
                    for h in range(2):
                        nc.tensor.matmul(
                            ps[:, h * 512 : (h + 1) * 512],
                            e_sb[:, b * NLANE : (b + 1) * NLANE],
                            ypt[0:C, bl * T + h * 512 : bl * T + (h + 1) * 512],
                            start=True,
                            stop=True,
                        )
                    gb = gb_pool.tile([NLANE, T], bf, tag="gb")
                    nc.scalar.copy(gb[:], ps[:])
                    nc.scalar.dma_start(gbd.ap()[b : b + 1, :], gb[:])

            nc.sync.dma_start(ylab[:], gbd.ap()[:, :])

            # ---- lane scans ----
            if phase == "gather":
                res_g = fin_pool.tile([BPC, 1], f32, tag="res", name="res_g")
                nc.vector.tensor_scalar(res_g[:], m_sb[:, 0:1], 1.0, 0.0, MULT, ADD)
                nc.sync.dma_start(out_d.ap()[:, :], res_g[:])

            def lane_view(l):
                return ylab[:, l * T : (l + 1) * T]

            pblank = lane_view(L)

            if phase == "gather":
                lanes_range = range(0)
            else:
                lanes_range = range(1, S)
                # lane 0 is a pure cumprod: state = (pblank * state) [bypass d1]
                nc.vector.tensor_tensor_scan(
                    atiles[0][:, 1 : T + 1], pblank, pblank, 1.0,
                    op0=MULT, op1=mybir.AluOpType.bypass,
                )
                if debug_taps:
                    nc.sync.dma_start(dbg_lanes.ap()[0], atiles[0][:])
            prev2, prev = None, atiles[0]
            for s in lanes_range:
                an = atiles[1 + (s - 1) % 3]
                if s % 2 == 0:
                    d0, d1 = prev[:, 0:T], pblank
                elif s == 1:
                    w = w_pool.tile([BPC, T], bf, tag="w")
                    nc.vector.tensor_scalar_mul(w[:], prev[:, 0:T], float(V2))
                    d0, d1 = w[:], lane_view(0)
                else:
                    l = (s - 1) // 2
                    # wm = (m~ * V2) * sigma[s-2] on ACT: depends on the scan
                    # two lanes back, so it hides under scan(s-1) on DVE.
                    wm = w_pool.tile([BPC, T], bf, tag="wm", name=f"wm{s}")
                    nc.scalar.mul(wm[:], prev2[:, 0:T], m_sb[:, l : l + 1])
                    w = w_pool.tile([BPC, T], bf, tag="w")
                    nc.vector.tensor_scalar_mul(w[:], prev[:, 0:T], float(V2))
                    d0t = d_pool.tile([BPC, T], bf, tag="d0")
                    nc.vector.tensor_tensor(d0t[:], w[:], wm[:], op=ADD)
                    d0, d1 = d0t[:], lane_view(l)
                nc.vector.tensor_tensor_scan(
                    an[:, 1 : T + 1], d0, d1, 0.0, op0=ADD, op1=MULT
                )
                if debug_taps:
                    nc.sync.dma_start(dbg_lanes.ap()[s], an[:])
                prev2, prev = prev, an

            # ---- final ----
            if phase != "gather":
                x = fin_pool.tile([BPC, 1], f32, tag="x")
                nc.vector.tensor_tensor(
                    x[:], prev[:, T : T + 1], prev2[:, T : T + 1], op=ADD
                )
                lnx = fin_pool.tile([BPC, 1], f32, tag="lnx")
                # ACT Ln LUT range is +-2^64; our x can reach ~e^76, so fold a
                # 2^-64 prescale into the op (compensated in K_FIN).
                nc.scalar.activation(
                    lnx[:], x[:], mybir.ActivationFunctionType.Ln, scale=float(2.0**-64)
                )
                res = fin_pool.tile([BPC, 1], f32, tag="res")
                nc.vector.tensor_scalar(res[:], lnx[:], -1.0, K_FIN, MULT, ADD)
                nc.sync.dma_start(out_d.ap()[:, :], res[:])

    nc.compile()
    return nc


def _host_inputs(y_true, y_pred):
    """Per-core input maps."""
    bf16 = ml_dtypes.bfloat16
    in_maps = []
    for i in range(NCORES):
        sl = slice(i * BPC, (i + 1) * BPC)
        lab = np.asarray(y_true[sl], dtype=np.int64)
        onehot = np.zeros((C, BPC * NLANE), dtype=bf16)
        for b in range(BPC):
            onehot[lab[b], b * NLANE + np.arange(L)] = bf16(1.0)
            onehot[BLANK, b * NLANE + L] = bf16(1.0)
        mv2 = np.zeros((BPC, L), dtype=np.float32)
        mv2[:, 1:] = (lab[:, 1:] != lab[:, :-1]).astype(np.float32) * V2
        in_maps.append(
            {
                "y_pred": np.ascontiguousarray(np.asarray(y_pred[sl], np.float32)),
                "onehot": onehot,
                "mv2": mv2,
                "c8pair": np.stack([C8, C8_EPS], axis=1),
            }
        )
    return in_maps


def kernel(y_true, y_pred):
    global _PROGRAM
    from concourse.bass_utils import run_bass_kernel_spmd

    y_true = np.asarray(y_true)
    y_pred = np.asarray(y_pred, dtype=np.float32)
    if _PROGRAM is None:
        _PROGRAM = _build_program()
    in_maps = _host_inputs(y_true, y_pred)
    r = run_bass_kernel_spmd(_PROGRAM, in_maps, list(range(NCORES)))
    out = np.concatenate([r.results[i]["out"] for i in range(NCORES)], axis=0)
    return out.astype(np.float32)

